# revision 15
# baseline (speedup 1.0000x reference)
"""Trainium2 Bass kernel (bf16 matmul operands, fp32 PSUM accumulation) for multi-head attention (nn_Attention_24764781428921).

Reference (fp32):
    q = heads(x @ Wq + bq); k = heads(x @ Wk + bk); v = heads(x @ Wv + bv)
    probs = softmax(q k^T / sqrt(1024)); ctx = probs @ v
    out = unheads(ctx) @ Wo + bo
with x [2, 2048, 1024], 16 heads, head_dim 64.

Sharding: DP=2 over batch x TP=4 over heads (4 heads / 256 channels per core).
Each core returns a partial [2048, 1024] = ctx_local @ Wo[local_rows]; the host
sums the 4 TP partials per batch and adds bo (the unshard/reduce step).

On-chip layout (per core):
  xT [1024, 2048]  (host pre-transposed)  -> SBUF [128, 8ct, 2048]
  qT/kT = W^T xT + b : [256, 2048] as [128, 2dt, 2048]   (channel on partitions)
  v     = xT^T Wv    : [2048, 256] -> vaug [128, 16jt, 4h, 65] (col 64 = ones)
  s^T(h, jt, ib) [128j, 512i] = kT_h[:, jt]^T-free q: matmul(lhsT=kT slice, rhs=qT slice)
     head pairs row-packed (K=64 at partition offsets 0/64 run concurrently)
  exp on ScalarE with scale=1/32 folded in (no max subtraction: |s|<~2)
  ctx^T+sums [65, 512] accumulated over 16 jt (ones row gives softmax denom)
  normalize: recip(sums) -> DRAM -> partition-broadcast [64,512]; mul + bias bv
  out[i, o] partial: matmul(lhsT=ctxn^T [c,128i], rhs=Wo [c, 512o]) accum 2 ct
"""

import numpy as np

HID = 1024
N = 2048
DL = 256          # local channels per core (4 heads x 64)
NHEAD = 4         # local heads
HD = 64
CT = HID // 128   # 8 c-tiles
DT = DL // 128    # 2 d-tiles
JT = N // 128     # 16 j-tiles
IB = N // 512     # 4 i-blocks
SCALE = 1.0 / 32.0  # 1/sqrt(1024)

_prog_cache = {}


def build_program(reps=1, loop_reps=None):
    import concourse.bass as bass
    import concourse.mybir as mybir
    import concourse.tile as tile
    from concourse import bacc

    F32 = mybir.dt.float32
    F32R = mybir.dt.bfloat16  # matmul operand dtype: bf16 gets FWL weight loads + 1 cyc/row
    AF = mybir.ActivationFunctionType

    def mm(out, lhsT, rhs, **kw):
        nc.tensor.matmul(out, lhsT=lhsT, rhs=rhs, **kw)

    nc = bacc.Bacc()
    xT = nc.dram_tensor("xT", [HID, N], F32R, kind="ExternalInput")
    wq0 = nc.dram_tensor("wq0", [128, CT * 128], F32R, kind="ExternalInput")
    wq1 = nc.dram_tensor("wq1", [128, CT * 128], F32R, kind="ExternalInput")
    wk0 = nc.dram_tensor("wk0", [128, CT * 128], F32R, kind="ExternalInput")
    wk1 = nc.dram_tensor("wk1", [128, CT * 128], F32R, kind="ExternalInput")
    wvh = nc.dram_tensor("wvh", [128, CT * DL], F32R, kind="ExternalInput")
    woh = nc.dram_tensor("woh", [128, DT * HID], F32R, kind="ExternalInput")
    aux = nc.dram_tensor("aux", [65, 640], F32R, kind="ExternalInput")
    ones = nc.dram_tensor("ones", [128, JT * NHEAD], F32R, kind="ExternalInput")
    bvb = nc.dram_tensor("bvb", [128, DL], F32, kind="ExternalInput")
    y = nc.dram_tensor("y", [N, HID], F32R, kind="ExternalOutput")

    with tile.TileContext(nc) as tc:
        with (
            tc.tile_pool(name="consts", bufs=1) as consts,
            tc.tile_pool(name="qkv_sb", bufs=1) as qkv_sb,
            tc.tile_pool(name="exp_sb", bufs=8) as exp_pool,
            tc.tile_pool(name="ctxn_sb", bufs=4) as ctxn_pool,
            tc.tile_pool(name="bcast_sb", bufs=4) as bcast_pool,
            tc.tile_pool(name="rec_sb", bufs=4) as rec_pool,
            tc.tile_pool(name="out_sb", bufs=6) as out_pool,
            tc.tile_pool(name="scr_dram", bufs=4, space="DRAM") as scr_pool,
        ):
            from contextlib import nullcontext
            loop_cm = tc.For_i(0, loop_reps, 1) if loop_reps is not None else nullcontext()
            with loop_cm:
              for rep in range(reps):
                  # ---- load constants (order matters: first k-proj matmul needs
                  # only wk[ct0] + xt[ct0], so emit those first on the SP HWDGE queue) ----

                  xt_sb = consts.tile([128, CT, N], F32R)
                  xt_r = xT[:, :].rearrange("(ct p) i -> p ct i", p=128)
                  wk_sb = consts.tile([128, 2, CT, 128], F32R)
                  wq_sb = consts.tile([128, 2, CT, 128], F32R)
                  wv_sb = consts.tile([128, CT, DL], F32R)
                  # coarse DMAs (each dma_start costs ~625ns of serialized
                  # HWDGE dispatch); host pre-transposes weights so every
                  # transfer is contiguous per partition; first k-matmul
                  # operands land first
                  nc.sync.dma_start(
                      out=wk_sb[:, 0, :, :],
                      in_=wk0[:, :].rearrange("p (ct d) -> p ct d", d=128))
                  nc.sync.dma_start(out=xt_sb[:, 0, 0:512], in_=xt_r[:, 0, 0:512])
                  nc.sync.dma_start(
                      out=wq_sb[:, 0, :, :],
                      in_=wq0[:, :].rearrange("p (ct d) -> p ct d", d=128))
                  nc.sync.dma_start(out=xt_sb[:, 0, 512:N], in_=xt_r[:, 0, 512:N])
                  nc.sync.dma_start(out=xt_sb[:, 1, :], in_=xt_r[:, 1, :])
                  nc.sync.dma_start(out=xt_sb[:, 2, :], in_=xt_r[:, 2, :])
                  for ct in range(3, CT):
                      nc.sync.dma_start(out=xt_sb[:, ct, :], in_=xt_r[:, ct, :])
                  aux_sb = consts.tile([65, 640], F32R)
                  nc.sync.dma_start(out=aux_sb[0:1, :], in_=aux[0:1, :])
                  nc.sync.dma_start(out=aux_sb[64:65, :], in_=aux[64:65, :])
                  onesr_sb = aux_sb[:, 0:512]
                  bq2_sb = aux_sb[:, 512:640]
                  bvb_sb = consts.tile([128, DL], F32)
                  nc.sync.dma_start(out=bvb_sb, in_=bvb[:, :])
                  nc.sync.dma_start(
                      out=wv_sb, in_=wvh[:, :].rearrange("p (ct d) -> p ct d", d=DL))
                  nc.sync.dma_start(
                      out=wk_sb[:, 1, :, :],
                      in_=wk1[:, :].rearrange("p (ct d) -> p ct d", d=128))
                  nc.sync.dma_start(
                      out=wq_sb[:, 1, :, :],
                      in_=wq1[:, :].rearrange("p (ct d) -> p ct d", d=128))
                  wo_sb = consts.tile([128, DT, HID], F32R)
                  nc.sync.dma_start(
                      out=wo_sb, in_=woh[:, :].rearrange("p (ct o) -> p ct o", o=HID))

                  qT = qkv_sb.tile([128, DT, N], F32R, tag="qT")
                  kT = qkv_sb.tile([128, DT, N], F32R, tag="kT")
                  vaug = qkv_sb.tile([128, JT, NHEAD, HD + 1], F32R, tag="vaug")
                  nc.sync.dma_start(
                      out=vaug[:, :, :, HD:HD + 1],
                      in_=ones[:, :].rearrange("p (jt h) -> p jt h", h=NHEAD)[:, :, :, None],
                  )

                  def proj_dt(w_sb, dest, dt, pool, nblk, pname, is_q):
                      """One d-tile of a q/k projection, c-accumulated in PSUM."""
                      for ib0 in range(0, IB, nblk):
                          pss = [
                              pool.tile([128, 512], F32, tag="pss",
                                        name=f"r{rep}_{pname}_{dt}_{ib0 + i}")
                              for i in range(nblk)
                          ]
                          for ct in range(CT):
                              for i in range(nblk):
                                  mm(
                                      pss[i],
                                      w_sb[:, dt, ct, :],
                                      xt_sb[:, ct, (ib0 + i) * 512:(ib0 + i + 1) * 512],
                                      start=(ct == 0),
                                      stop=(ct == CT - 1 and not is_q),
                                  )
                          for i in range(nblk):
                              ib = ib0 + i
                              if is_q:
                                  mm(pss[i], bq2_sb[dt * 64:dt * 64 + 1, :], onesr_sb[dt * 64:dt * 64 + 1, :],
                                     start=False, stop=True)
                              if is_q:
                                  nc.scalar.activation(
                                      dest[:, dt, ib * 512:(ib + 1) * 512],
                                      pss[i], AF.Copy)
                              else:
                                  nc.vector.tensor_copy(
                                      out=dest[:, dt, ib * 512:(ib + 1) * 512],
                                      in_=pss[i],
                                  )

                  def emit_v(jt, ps_v):
                      """v[j, d] = xT^T Wv for one j-tile (bias folded in after attn)."""
                      psv = ps_v.tile([128, DL], F32, tag="psv", name=f"r{rep}_psv_{jt}")
                      for ct in range(CT):
                          mm(
                              psv,
                              xt_sb[:, ct, jt * 128:(jt + 1) * 128],
                              wv_sb[:, ct, :],
                              start=(ct == 0),
                              stop=(ct == CT - 1),
                          )
                      nc.vector.tensor_add(
                          vaug[:, jt, :, 0:HD],
                          psv.rearrange("p (h d) -> p h d", h=NHEAD),
                          bvb_sb.rearrange("p (h d) -> p h d", h=NHEAD),
                      )

                  def attn_block(ib, hp, ctxn, ps_s, ps_ctx, ps_bc, v_fn=None):
                      """Scores + exp + ctx accumulate + normalize for one head pair.

                      Emission is software-pipelined one jt ahead so the PE stream
                      never waits in-line on the exp of the current jt. v_fn(jt)
                      optionally interleaves the v projection (phase B fusion).
                      """
                      isl = slice(ib * 512, (ib + 1) * 512)
                      ctx_ps = [
                          ps_ctx.tile([HD + 1, 512], F32, tag="ctx",
                                      name=f"r{rep}_ctx_{ib}_{hp}_{u}")
                          for u in range(2)
                      ]
                      exs = {}
                      for jt in range(JT):
                          if v_fn is not None:
                              v_fn(jt)
                          s_ps = ps_s.tile([128, 2, 512], F32, tag="s", name=f"r{rep}_s_{ib}_{hp}_{jt}")
                          for u in range(2):
                              poff = u * HD
                              mm(
                                  s_ps[:, u, :],
                                  kT[poff:poff + HD, hp, jt * 128:(jt + 1) * 128],
                                  qT[poff:poff + HD, hp, isl],
                                  start=True,
                                  stop=True,
                              )
                          ex = exp_pool.tile([128, 2, 512], F32R, tag="ex", name=f"r{rep}_ex_{ib}_{hp}_{jt}")
                          nc.scalar.activation(ex, s_ps, AF.Exp, scale=SCALE)
                          exs[jt] = ex
                          if jt > 0:
                              emit_ctx(ib, hp, ctx_ps, exs.pop(jt - 1), jt - 1)
                      emit_ctx(ib, hp, ctx_ps, exs.pop(JT - 1), JT - 1)

                      rec = rec_pool.tile([65, 512], F32R, tag="rec", name=f"r{rep}_rec_{ib}_{hp}")
                      with nc.allow_low_precision(reason="f32r is fp32-width; matmul input rounding"):
                          nc.vector.reciprocal(rec[0:1, :], ctx_ps[0][HD:HD + 1, :])
                          nc.vector.reciprocal(rec[64:65, :], ctx_ps[1][HD:HD + 1, :])
                      # partition-broadcast: one K=1 outer product per head,
                      # both into one psum bank (u1 at column position 64)
                      bc_ps = ps_bc.tile([128, 512], F32, tag="bc", name=f"r{rep}_bcps_{ib}_{hp}")
                      for u in range(2):
                          poff = u * HD
                          mm(bc_ps[poff:poff + HD, :],
                             onesr_sb[poff:poff + 1, 0:HD],
                             rec[poff:poff + 1, :],
                             start=True, stop=True)
                      bc = bcast_pool.tile([128, 512], F32, tag="bc",
                                           name=f"r{rep}_bc_{ib}_{hp}")
                      nc.vector.tensor_copy(out=bc, in_=bc_ps)
                      for u in range(2):
                          poff = u * HD
                          nc.vector.tensor_mul(
                              ctxn[poff:poff + HD, hp, :], ctx_ps[u][0:HD, :],
                              bc[poff:poff + HD, :]
                          )


                  def emit_ctx(ib, hp, ctx_ps, ex, jt):
                      for u in range(2):
                          h = 2 * hp + u
                          mm(
                              ctx_ps[u],
                              vaug[:, jt, h, :],
                              ex[:, u, :],
                              start=(jt == 0),
                              stop=(jt == JT - 1),
                          )

                  def wo_steps(ib, ctxn, pools):
                      """Per-jt filler steps for Wo(ib): 12 small emissions."""
                      for step in _wo_emit(ib, ctxn, pools, as_steps=True):
                          yield step

                  def emit_wo(ib, ctxn, pools, ct_split=False):
                      for _ in _wo_emit(ib, ctxn, pools, ct_split=ct_split):
                          pass

                  def _wo_emit(ib, ctxn, pools, ct_split=False, as_steps=False):
                      pairs = {}
                      gen = [0]
                      def wo_alloc(it):
                          pool, tag = pools[gen[0] % len(pools)]
                          gen[0] += 1
                          pairs[it] = [
                              pool.tile([128, 512], F32, tag=tag,
                                        name=f"r{rep}_pso_{ib}_{it}_{ot}")
                              for ot in range(2)
                          ]
                      def wo_mm(it, ct):
                          for ot in range(2):
                              mm(
                                  pairs[it][ot],
                                  ctxn[:, ct, it * 128:(it + 1) * 128],
                                  wo_sb[:, ct, ot * 512:(ot + 1) * 512],
                                  start=(ct == 0),
                                  stop=(ct == DT - 1),
                                  skip_group_check=ct_split,
                              )
                      def wo_fin(it):
                          ob = out_pool.tile([128, 2, 512], F32R, tag="ob",
                                             name=f"r{rep}_ob_{ib}_{it}")
                          for ot in range(2):
                              nc.vector.tensor_copy(out=ob[:, ot, :], in_=pairs[it][ot])
                          nc.gpsimd.dma_start(
                              out=y[ib * 512 + it * 128:ib * 512 + (it + 1) * 128, :],
                              in_=ob.rearrange("p a b -> p (a b)"),
                          )
                      if as_steps:
                          for it in range(4):
                              def s1(it=it):
                                  wo_alloc(it)
                                  wo_mm(it, 0)
                              def s2(it=it):
                                  wo_mm(it, 1)
                              def s3(it=it):
                                  wo_fin(it)
                              yield s1
                              yield s2
                              yield s3
                      elif ct_split:
                          for it in (0, 1):
                              wo_alloc(it)
                              wo_mm(it, 0)
                          yield  # final normalize goes here
                          for it in (0, 1):
                              wo_mm(it, 1)
                              wo_fin(it)
                          for it in (2, 3):
                              wo_alloc(it)
                              wo_mm(it, 0)
                              wo_mm(it, 1)
                              wo_fin(it)
                      else:
                          for it in range(4):
                              wo_alloc(it)
                              wo_mm(it, 0)
                              wo_mm(it, 1)
                              wo_fin(it)
                          yield

                  # ---- phase A: dt0 projections, ct-outer with k/q interleaved
                  # across 8 psums so the PE tracks the per-ct xt DMA arrival ----
                  with tc.tile_pool(name="ps_proj", bufs=8, space="PSUM") as ps_proj:
                      pps = {}
                      for pn in range(2):
                          for ib in range(IB):
                              pps[pn, ib] = ps_proj.tile(
                                  [128, 512], F32, tag="pss",
                                  name=f"r{rep}_upA_{pn}_{ib}")
                      for ct in range(CT):
                          for pn, w_sb in ((0, wk_sb), (1, wq_sb)):
                              for ib in range(IB):
                                  mm(
                                      pps[pn, ib],
                                      w_sb[:, 0, ct, :],
                                      xt_sb[:, ct, ib * 512:(ib + 1) * 512],
                                      start=(ct == 0),
                                      stop=(ct == CT - 1 and pn == 0),
                                  )
                      # bq folded in as a K=1 ones-row matmul (bk dropped:
                      # q.bk is constant per softmax row); evacuations are
                      # plain copies split across DVE and GpSimd
                      for pn, dest in ((0, kT), (1, qT)):
                          for ib in range(IB):
                              isl = slice(ib * 512, (ib + 1) * 512)
                              if pn == 1:
                                  mm(pps[pn, ib], bq2_sb[0:1, :], onesr_sb[0:1, :],
                                     start=False, stop=True)
                              if pn == 0:
                                  nc.vector.tensor_copy(out=dest[:, 0, isl], in_=pps[pn, ib])
                              else:
                                  nc.scalar.activation(dest[:, 0, isl], pps[pn, ib], AF.Copy)

                  # ---- phase B: hp0 attention (v fused into first block), dt1
                  # projections slipped between, then hp1 blocks + Wo ----
                  with (
                      tc.tile_pool(name="ps_s", bufs=2, space="PSUM") as ps_s,
                      tc.tile_pool(name="ps_ctx", bufs=2, space="PSUM") as ps_ctx,
                      tc.tile_pool(name="ps_bc", bufs=1, space="PSUM") as ps_bc,
                  ):
                      ctxns = {
                          ib: ctxn_pool.tile([128, DT, 512], F32R, tag="ctxn",
                                             name=f"r{rep}_ctxn_{ib}")
                          for ib in range(IB)
                      }
                      with tc.tile_pool(name="ps_v", bufs=1, space="PSUM") as ps_v:
                          attn_block(0, 0, ctxns[0], ps_s, ps_ctx, ps_bc,
                                     v_fn=lambda jt: emit_v(jt, ps_v))
                      for ib in range(1, IB):
                          attn_block(ib, 0, ctxns[ib], ps_s, ps_ctx, ps_bc)
                      with tc.tile_pool(name="ps_projB", bufs=1, space="PSUM") as ps_projB:
                          proj_dt(wk_sb, kT, 1, ps_projB, 1, "k", False)
                          proj_dt(wq_sb, qT, 1, ps_projB, 1, "q", True)
                      with tc.tile_pool(name="ps_o", bufs=1, space="PSUM") as ps_o:
                          for ib in range(IB):
                              attn_block(ib, 1, ctxns[ib], ps_s, ps_ctx, ps_bc)
                              if 1 <= ib <= 2:
                                  # Wo for the PREVIOUS block: its ctxn has
                                  # been ready for a whole block -> no
                                  # normalize-latency stall
                                  emit_wo(ib - 1, ctxns[ib - 1], [(ps_o, "pso")])
                          # tail: Wo(2)+Wo(3) rotate through ps_o and the
                          # now-dead ctx/bc slots for an effective ring of 3
                          tail_pools = [(ps_o, "pso"), (ps_ctx, "ctx"),
                                        (ps_bc, "bc")]
                          emit_wo(2, ctxns[2], tail_pools)
                          emit_wo(3, ctxns[3], tail_pools)

    nc.compile()
    return nc


import ml_dtypes as _mld
BF16 = _mld.bfloat16
_ONES = np.ones((128, JT * NHEAD), dtype=BF16)


def _wslice(w_l, dt):
    # [1024, 256] local weight -> [128, CT*128] (partition-contiguous dt half)
    return np.ascontiguousarray(
        w_l[:, dt * 128:(dt + 1) * 128].reshape(CT, 128, 128)
        .transpose(1, 0, 2).reshape(128, CT * 128))


def _aux(bq_loc):
    # [65, 640]: cols 0:512 ones rows(0,64); cols 512:640 bq rows (dt0, dt1)
    out = np.zeros((65, 640), dtype=BF16)
    out[0, 0:512] = 1
    out[64, 0:512] = 1
    r = bq_loc.reshape(DT, 128).astype(BF16)
    out[0, 512:640] = r[0]
    out[64, 512:640] = r[1]
    return out


def _prepare_core_inputs(x, Wq, bq, Wk, bk, Wv, bv, Wo):
    """Shard: core = b*4 + g; batch b, head-group g (channels 256g..256g+256)."""
    in_maps = []
    xTs = [np.ascontiguousarray(np.asarray(x[b]).T.astype(BF16)) for b in range(2)]
    for core in range(8):
        b, g = core // 4, core % 4
        cols = slice(g * DL, (g + 1) * DL)
        wq_l, wk_l, wv_l = (W[:, cols].astype(BF16) for W in (Wq, Wk, Wv))
        in_maps.append({
            "xT": xTs[b],
            "wq0": _wslice(wq_l, 0), "wq1": _wslice(wq_l, 1),
            "wk0": _wslice(wk_l, 0), "wk1": _wslice(wk_l, 1),
            "wvh": np.ascontiguousarray(
                Wv[:, cols].astype(BF16).reshape(CT, 128, DL)
                .transpose(1, 0, 2).reshape(128, CT * DL)),
            "woh": np.ascontiguousarray(
                Wo[cols, :].astype(BF16).reshape(DT, 128, HID)
                .transpose(1, 0, 2).reshape(128, DT * HID)),
            "aux": _aux(bq[cols]),
            "bvb": np.ascontiguousarray(np.tile(bv[cols][None, :], (128, 1))),
            "ones": _ONES,
        })
    return in_maps


def kernel(x, Wq, bq, Wk, bk, Wv, bv, Wo, bo, _trace=False, _results_box=None):
    from concourse.bass_utils import run_bass_kernel_spmd

    x = np.asarray(x, dtype=np.float32)
    args = [np.asarray(a, dtype=np.float32) for a in (Wq, bq, Wk, bk, Wv, bv, Wo, bo)]
    Wq, bq, Wk, bk, Wv, bv, Wo, bo = args

    if "nc" not in _prog_cache:
        _prog_cache["nc"] = build_program()
    nc = _prog_cache["nc"]

    in_maps = _prepare_core_inputs(x, Wq, bq, Wk, bk, Wv, bv, Wo)
    res = run_bass_kernel_spmd(nc, in_maps, core_ids=list(range(8)), trace=_trace)
    if _results_box is not None:
        _results_box.append(res)
    parts = [np.asarray(r["y"], dtype=np.float32) for r in res.results]
    out = np.empty((2, N, HID), dtype=np.float32)
    for b in range(2):
        out[b] = parts[4 * b] + parts[4 * b + 1] + parts[4 * b + 2] + parts[4 * b + 3]
    out += bo
    return out

